# revision 1
# baseline (speedup 1.0000x reference)
"""CRF loss (forward-algorithm partition function minus gold path score) on 8 Trainium2 cores.

Algorithm
---------
reference: fv_{t}[j] = logsumexp_i(fv_{t-1}[i] + trans[j,i]) + obs[t,j], fv_0 = 0,
loss = logsumexp(fv_T) - gold.

In the exp domain the recurrence is linear-positive:
    w_t = diag(exp(obs_t - ALPHA)) . E . w_{t-1},   E = exp(trans)
Products of positive matrices forget direction geometrically (Birkhoff
contraction; empirically bit-exact fp32 convergence in < 10 steps for this
data distribution). So the T=32768-step chain is split into 8*R independent
sub-chunks of L steps, each "speculatively" warmed up with B burn-in steps
from the all-ones vector. Per sub-chunk q we record log(sum(w)) right after
its burn-in (time s_q) and at its end (time e_q = s_{q+1}); scale factors of
the speculative trajectory cancel inside the difference, and the differences
telescope across sub-chunks:
    logsumexp(fv_T) = sum_q [log sig_e(q) - log sig_s(q)] + T*ALPHA + log(512)
Sub-chunk q=0 is re-initialized to the exact all-ones state at time 0.

Each core runs R=256 sub-chunk states in lock-step: one inner step is a
512x512 @ 512x256 matmul (fp32r on the PE) plus an elementwise multiply by
exp(obs - ALPHA) (DVE). The obs slice is laid out host-side in an
"i-major" order (c, r') with source column = 16*r' + c so every per-step
operand is a contiguous [128, 256] slice — no strided engine reads.

gold = sum_i trans[tags[i+1],tags[i]] + observes[tags[i+1], i] is computed
with per-partition indirect-DMA element gathers (one offset per contiguous
dest run is the HW contract), sharded over cores and overlapped with the
forward loop on the otherwise-idle gpsimd engine.
"""

import sys

sys.path.insert(0, "/opt/trn_rl_repo")

import numpy as np

import concourse.bacc as bacc
import concourse.bass as bass
import concourse.mybir as mybir
import concourse.tile as tile
from concourse.bass import IndirectOffsetOnAxis
from concourse.bass_types import AP
from concourse.bass_utils import run_bass_kernel_spmd

K = 512          # tagset size
T = 32768        # sequence length
NCORES = 8
R = 256          # parallel sub-chunk states per core
L = 16           # owned steps per sub-chunk
B = 8            # burn-in steps per sub-chunk
ALPHA = 7.25     # fixed per-step log-gain shift (keeps fp32 state in range)
NSTEP = B + L    # inner steps per core
S = B + R * L    # per-core obs slice length (4112)
RW = R + 2       # r' width of the i-major layout (258)
S2 = 16 * RW     # padded/packed slice length (4128)
GN = T // NCORES                # gold indices per core (4096)
GIT = GN // 128                 # gold free dim (32)

F32 = mybir.dt.float32
F32R = mybir.dt.float32r
I32 = mybir.dt.int32

assert NCORES * R * L == T and NSTEP <= 32 and L == 16


def _build_nc():
    nc = bacc.Bacc("TRN2", target_bir_lowering=False, debug=False)

    # obs is the per-core slice in i-major packed layout:
    # obs[k, c*RW + r'] = obs_slice[k, 16*r' + c]
    obs = nc.dram_tensor("obs", [K, S2], F32, kind="ExternalInput")
    trans = nc.dram_tensor("trans", [K, K], F32, kind="ExternalInput")
    maskA = nc.dram_tensor("maskA", [4, 128], F32, kind="ExternalInput")
    maskB = nc.dram_tensor("maskB", [4, 128], F32, kind="ExternalInput")
    nxt = nc.dram_tensor("nxt", [GN], I32, kind="ExternalInput")
    cur = nc.dram_tensor("cur", [GN], I32, kind="ExternalInput")
    qg = nc.dram_tensor("qg", [128, GIT], I32, kind="ExternalInput")
    gmask = nc.dram_tensor("gmask", [128, GIT], F32, kind="ExternalInput")
    out = nc.dram_tensor("out", [1, 4], F32, kind="ExternalOutput")

    with tile.TileContext(nc) as tc:
        with (
            tc.tile_pool(name="const", bufs=1) as cpool,
            tc.tile_pool(name="etp", bufs=1) as etpool,
            tc.tile_pool(name="trp", bufs=1) as trpool,
            tc.tile_pool(name="dxp", bufs=1) as dxpool,
            tc.tile_pool(name="raw", bufs=2) as rawpool,
            tc.tile_pool(name="wp", bufs=2) as wpool,
            tc.tile_pool(name="gld", bufs=1) as gpool,
            tc.tile_pool(name="ups", bufs=1, space="PSUM") as upool,
            tc.tile_pool(name="sps", bufs=1, space="PSUM") as spool,
        ):
            # -------- gold gather inputs first: tiny DMAs ahead of the big
            # obs stream so the gpsimd gathers can start immediately --------
            nxt_sb = gpool.tile([128, GIT], I32, tag="nxt_sb", name="nxt_sb")
            nc.sync.dma_start(nxt_sb[:],
                              nxt[:].rearrange("(it p) -> p it", p=128))
            cur_sb = gpool.tile([128, GIT], I32, tag="cur_sb", name="cur_sb")
            nc.sync.dma_start(cur_sb[:],
                              cur[:].rearrange("(it p) -> p it", p=128))
            qg_sb = gpool.tile([128, GIT], I32, tag="qg_sb", name="qg_sb")
            nc.sync.dma_start(qg_sb[:], qg[:, :])
            gm_sb = gpool.tile([128, GIT], F32, tag="gm_sb", name="gm_sb")
            nc.sync.dma_start(gm_sb[:], gmask[:, :])

            offs_tr = gpool.tile([128, GIT], I32, tag="offs_tr", name="offs_tr")
            nc.vector.tensor_scalar(offs_tr[:], nxt_sb[:], K, None,
                                    op0=mybir.AluOpType.mult)
            nc.vector.tensor_add(offs_tr[:], offs_tr[:], cur_sb[:])
            offs_ob = gpool.tile([128, GIT], I32, tag="offs_ob", name="offs_ob")
            nc.vector.tensor_scalar(offs_ob[:], nxt_sb[:], S2, None,
                                    op0=mybir.AluOpType.mult)
            nc.vector.tensor_add(offs_ob[:], offs_ob[:], qg_sb[:])

            # HW indirect DMA consumes one offset per contiguous dest run
            # (row): gather one element per partition per instruction.
            g_tr = gpool.tile([128, GIT], F32, tag="g_tr", name="g_tr")
            g_ob = gpool.tile([128, GIT], F32, tag="g_ob", name="g_ob")
            trans_flat = trans[:, :].rearrange("(o a) b -> o (a b)", o=1)
            obs_flat = obs[:, :].rearrange("(o a) b -> o (a b)", o=1)
            for it in range(GIT):
                nc.gpsimd.indirect_dma_start(
                    g_tr[:, it:it + 1], None, trans_flat,
                    IndirectOffsetOnAxis(ap=offs_tr[:, it:it + 1], axis=1))
                nc.gpsimd.indirect_dma_start(
                    g_ob[:, it:it + 1], None, obs_flat,
                    IndirectOffsetOnAxis(ap=offs_ob[:, it:it + 1], axis=1))

            # ---------------- constants ----------------
            iota_p = cpool.tile([128, 1], F32, tag="iota_p", name="iota_p")
            nc.gpsimd.iota(iota_p[:], pattern=[[0, 1]], base=0,
                           channel_multiplier=1,
                           allow_small_or_imprecise_dtypes=True)
            iota_f = cpool.tile([128, 128], F32, tag="iota_f", name="iota_f")
            nc.gpsimd.iota(iota_f[:], pattern=[[1, 128]], base=0,
                           channel_multiplier=0,
                           allow_small_or_imprecise_dtypes=True)
            ident = cpool.tile([128, 128], F32, tag="ident", name="ident")
            nc.vector.tensor_scalar(ident[:], iota_f[:], iota_p[:], None,
                                    op0=mybir.AluOpType.is_equal)
            ones_f = cpool.tile([128, 1], F32, tag="ones_f", name="ones_f")
            nc.vector.memset(ones_f[:], 1.0)
            ones = cpool.tile([128, 1], F32R, tag="ones", name="ones")
            nc.vector.tensor_copy(ones[:], ones_f[:])
            biasE = cpool.tile([128, 1], F32, tag="biasE", name="biasE")
            nc.vector.memset(biasE[:], -ALPHA)
            mA = cpool.tile([128, 4], F32, tag="mA", name="mA")
            nc.sync.dma_start(mA[:], maskA[:, :].rearrange("a b -> b a"))
            mB = cpool.tile([128, 4], F32, tag="mB", name="mB")
            nc.sync.dma_start(mB[:], maskB[:, :].rearrange("a b -> b a"))

            # ---------------- E^T = exp(trans)^T ----------------
            tr = [trpool.tile([128, K], F32, tag=f"tr{jt}", name=f"tr{jt}") for jt in range(4)]
            for jt in range(4):
                (nc.sync if jt % 2 == 0 else nc.scalar).dma_start(
                    tr[jt][:], trans[128 * jt:128 * (jt + 1), :])
            # et[ki][i_local, j] = exp(trans[j, 128*ki + i_local])
            et = [etpool.tile([128, K], F32R, tag=f"et{ki}", name=f"et{ki}") for ki in range(4)]
            for ki in range(4):
                for jt in range(4):
                    pt = upool.tile([128, 128], F32, tag=f"u{jt}", name=f"u{jt}")
                    nc.tensor.transpose(pt[:], tr[jt][:, 128 * ki:128 * (ki + 1)],
                                        ident[:])
                    nc.scalar.activation(et[ki][:, 128 * jt:128 * (jt + 1)], pt[:],
                                         mybir.ActivationFunctionType.Exp)

            # ---------------- obs slices: DMA + exp into i-major dexp ----------------
            # chunk g covers packed cols [1032*g, 1032*(g+1)) = c in [4g, 4g+4)
            CH = S2 // 4
            dexp = [dxpool.tile([128, S2], F32, tag=f"dexp{jt}", name=f"dexp{jt}")
                    for jt in range(4)]
            for g in range(4):
                for jt in range(4):
                    rawt = rawpool.tile([128, CH], F32, tag=f"raw{jt}", name=f"raw{jt}")
                    dma_eng = nc.sync if jt % 2 == 0 else nc.scalar
                    dma_eng.dma_start(
                        rawt[:],
                        obs[128 * jt:128 * (jt + 1), CH * g:CH * (g + 1)])
                    nc.scalar.activation(
                        dexp[jt][:, CH * g:CH * (g + 1)], rawt[:],
                        mybir.ActivationFunctionType.Exp, bias=biasE[:])

            # ---------------- state init ----------------
            w = [wpool.tile([128, R], F32R, tag=f"w{kt}", name=f"w{kt}") for kt in range(4)]
            for kt in range(4):
                nc.vector.tensor_copy(w[kt][:], ones_f[:].to_broadcast((128, R)))

            ls_sb = cpool.tile([1, R], F32, tag="ls_sb", name="ls_sb")
            le_sb = cpool.tile([1, R], F32, tag="le_sb", name="le_sb")

            # ---------------- main recurrence ----------------
            for i in range(1, NSTEP + 1):
                qq, cc = (i - 1) // 16, (i - 1) % 16

                u = [None] * 4
                for jt in range(4):
                    u[jt] = upool.tile([128, R], F32, tag=f"u{jt}", name=f"u{jt}")
                    for kt in range(4):
                        nc.tensor.matmul(
                            u[jt][:],
                            et[kt][:, 128 * jt:128 * (jt + 1)],
                            w[kt][:],
                            start=(kt == 0), stop=(kt == 3))

                wn = [wpool.tile([128, R], F32R, tag=f"w{kt}", name=f"w{kt}") for kt in range(4)]
                for jt in range(4):
                    nc.vector.tensor_mul(wn[jt][:], u[jt][:],
                                         dexp[jt][:, cc * RW + qq:cc * RW + qq + R])
                w = wn

                if i == B:
                    # reinit sub-chunk state column 0 (core 0 only via masks)
                    for jt in range(4):
                        nc.vector.tensor_mul(w[jt][:, 0:1], w[jt][:, 0:1],
                                             mA[:, jt:jt + 1].bitcast(F32R))
                        nc.vector.tensor_add(w[jt][:, 0:1], w[jt][:, 0:1],
                                             mB[:, jt:jt + 1].bitcast(F32R))
                if i == B or i == NSTEP:
                    sig = spool.tile([1, R], F32, tag="sig", name="sig")
                    for kt in range(4):
                        nc.tensor.matmul(sig[:], ones[:],
                                         w[kt][:],
                                         start=(kt == 0), stop=(kt == 3))
                    dst = ls_sb if i == B else le_sb
                    nc.scalar.activation(dst[:], sig[:],
                                         mybir.ActivationFunctionType.Ln)

            # ---------------- forward partial ----------------
            diff = cpool.tile([1, R], F32, tag="diff", name="diff")
            nc.vector.tensor_sub(diff[:], le_sb[:], ls_sb[:])
            fwd_red = cpool.tile([1, 1], F32, tag="fwd_red", name="fwd_red")
            nc.vector.tensor_reduce(fwd_red[:], diff[:],
                                    axis=mybir.AxisListType.X,
                                    op=mybir.AluOpType.add)

            # ---------------- gold tail ----------------
            # allocate from the W pool: the WAR dependency on the final
            # colsum readers keeps these DVE ops out of the loop's queue
            gsum = wpool.tile([128, GIT], F32, tag="w0", name="gsum")
            nc.vector.tensor_add(gsum[:], g_tr[:], g_ob[:])
            nc.vector.tensor_mul(gsum[:], gsum[:], gm_sb[:])
            gvec = wpool.tile([128, 1], F32, tag="w1", name="gvec")
            nc.vector.tensor_reduce(gvec[:], gsum[:],
                                    axis=mybir.AxisListType.X,
                                    op=mybir.AluOpType.add)
            gold_ps = spool.tile([1, 1], F32, tag="gold_ps", name="gold_ps")
            nc.tensor.matmul(gold_ps[:], gvec[:],
                             ones_f[:], start=True, stop=True)

            # ---------------- output ----------------
            out_sb = cpool.tile([1, 4], F32, tag="out_sb", name="out_sb")
            nc.vector.memset(out_sb[:], 0.0)
            nc.vector.tensor_copy(out_sb[:, 0:1], fwd_red[:])
            nc.vector.tensor_copy(out_sb[:, 1:2], gold_ps[:])
            nc.sync.dma_start(out[:, :], out_sb[:])

    nc.compile()
    return nc


_NC_CACHE = None


def _get_nc():
    global _NC_CACHE
    if _NC_CACHE is None:
        _NC_CACHE = _build_nc()
    return _NC_CACHE


def make_in_maps(observes, tags, transitions):
    observes = np.ascontiguousarray(np.asarray(observes, dtype=np.float32))
    transitions = np.ascontiguousarray(np.asarray(transitions, dtype=np.float32))
    tags = np.asarray(tags).astype(np.int32)
    assert observes.shape == (K, T) and transitions.shape == (K, K)

    in_maps = []
    for c in range(NCORES):
        lo = c * R * L - B
        sl = np.zeros((K, S2), np.float32)
        src_lo = max(lo, 0)
        sl[:, src_lo - lo:S] = observes[:, src_lo:c * R * L + R * L]
        # pack i-major: packed[k, cc*RW + r'] = sl[k, 16*r' + cc]
        packed = np.ascontiguousarray(
            sl.reshape(K, RW, 16).transpose(0, 2, 1).reshape(K, S2))

        mA = np.ones((4, 128), np.float32)
        mB = np.zeros((4, 128), np.float32)
        if c == 0:
            mA[:] = 0.0
            mB[:] = 1.0

        # gold index layout: flat arrays indexed by q; device views them as
        # [p, it] with q = it*128 + p.  packed position of slice col u:
        # (u % 16) * RW + u // 16
        idx = c * GN + np.arange(GN)
        q = (np.arange(GIT)[None, :] * 128 + np.arange(128)[:, None])
        u = q + B
        qgrid = ((u % 16) * RW + u // 16).astype(np.int32)
        gm = np.ones((128, GIT), np.float32)
        if c == NCORES - 1:
            gm[127, GIT - 1] = 0.0  # i = T-1 has no successor

        in_maps.append({
            "obs": packed,
            "trans": transitions,
            "maskA": mA,
            "maskB": mB,
            "nxt": np.ascontiguousarray(
                np.asarray(tags[np.minimum(idx + 1, T - 1)], np.int32)),
            "cur": np.ascontiguousarray(np.asarray(tags[idx], np.int32)),
            "qg": np.ascontiguousarray(qgrid),
            "gmask": gm,
        })
    return in_maps


def combine(results):
    fwd = 0.0
    gold = 0.0
    for c in range(NCORES):
        o = results[c]["out"]
        fwd += float(o[0, 0])
        gold += float(o[0, 1])
    loss = fwd + T * ALPHA + np.log(512.0) - gold
    return np.float32(loss)


def run(in_maps, trace=False):
    nc = _get_nc()
    res = run_bass_kernel_spmd(nc, in_maps, list(range(NCORES)), trace=trace)
    return res


def kernel(observes, tags, transitions, length):
    assert int(length) == T
    in_maps = make_in_maps(observes, tags, transitions)
    res = run(in_maps)
    return combine(res.results)



# revision 9
# speedup vs baseline: 1.4001x; 1.4001x over previous
"""CRF loss (forward-algorithm partition function minus gold path score) on 8 Trainium2 cores.

Algorithm
---------
reference: fv_{t}[j] = logsumexp_i(fv_{t-1}[i] + trans[j,i]) + obs[t,j], fv_0 = 0,
loss = logsumexp(fv_T) - gold.

In the exp domain the recurrence is linear-positive:
    w_t = diag(exp(obs_t - ALPHA)) . E . w_{t-1},   E = exp(trans)
Products of positive matrices forget direction geometrically (Birkhoff
contraction; for this data distribution the burn-in error is far below the
bf16 noise floor already at B=2-3 burn-in steps). The T=32768-step chain is
split into 8*R independent sub-chunks of L steps, each "speculatively" warmed
up with B burn-in steps from the all-ones vector. Per sub-chunk q we record
log(sum(w)) right after its burn-in (time s_q) and at its end (e_q = s_{q+1});
scale factors of the speculative trajectory cancel inside the difference, and
the differences telescope across sub-chunks:
    logsumexp(fv_T) = sum_q [log sig_e(q) - log sig_s(q)] + T*ALPHA + log(512)
Sub-chunk q=0 is re-initialized to the exact all-ones state at time 0.

Each core runs R=256 sub-chunk states in lock-step in bf16: one inner step is
a 512x512 @ 512x256 bf16 matmul on the PE (fp32 PSUM accumulation, FWL weight
loads) plus an elementwise multiply by exp(obs - ALPHA) on the DVE. The obs
slice is laid out host-side in an "i-major" order so every per-step operand
is a contiguous [128, 256] slice.

gold = sum_i trans[tags[i+1],tags[i]] + observes[tags[i+1], i] is computed
without any gathers: the host builds a count-mask `gmask` in the SAME layout
as the input blob ([packed obs | trans^T]); gold = sum(gmask * blob), done as
bf16 2x DVE multiplies + ScalarE accum_out reductions in ~600ns pieces that
are interleaved into the main loop's engine slack.
"""

import sys

sys.path.insert(0, "/opt/trn_rl_repo")

import numpy as np
import ml_dtypes

import concourse.bacc as bacc
import concourse.bass as bass
import concourse.mybir as mybir
import concourse.tile as tile
from concourse.bass_utils import run_bass_kernel_spmd

K = 512          # tagset size
T = 32768        # sequence length
NCORES = 8
R = 256          # parallel sub-chunk states per core
L = 16           # owned steps per sub-chunk
B = 3            # burn-in steps per sub-chunk
ALPHA = 7.25     # fixed per-step log-gain shift (keeps state in range)
NSTEP = B + L    # inner steps per core (19)
S = B + R * L    # valid cols of the per-core obs slice (4099)
RW = R + 2       # r' width of the i-major layout (258)
S2 = 16 * RW     # padded/packed slice length (4128)
BW = S2 + K      # blob width: packed obs ++ trans^T (4640)
GN = T // NCORES                # gold indices per core (4096)
# obs DMA chunks, in cc-block units (sum = 16)
CHUNKS = [1, 1, 2, 4, 8]
PIECE = 1032     # gold piece width (cols of a raw4/u4 tile)

F32 = mybir.dt.float32
BF16 = mybir.dt.bfloat16

assert NCORES * R * L == T and NSTEP <= 32 and L == 16


def _build_nc():
    nc = bacc.Bacc("TRN2", target_bir_lowering=False, debug=False)

    # blob row k = [packed obs slice row k (S2) | transT row k (K)]
    blob = nc.dram_tensor("blob", [K, BW], BF16, kind="ExternalInput")
    gmask = nc.dram_tensor("gmask", [K, BW], BF16, kind="ExternalInput")
    maskA = nc.dram_tensor("maskA", [128, 1], BF16, kind="ExternalInput")
    maskB = nc.dram_tensor("maskB", [128, 1], BF16, kind="ExternalInput")
    out = nc.dram_tensor("out", [1, 4], F32, kind="ExternalOutput")

    with tile.TileContext(nc) as tc:
        with (
            tc.tile_pool(name="const", bufs=1) as cpool,
            tc.tile_pool(name="etp", bufs=1) as etpool,
            tc.tile_pool(name="dxp", bufs=1) as dxpool,
            tc.tile_pool(name="raw", bufs=1) as rawpool,
            tc.tile_pool(name="gsc", bufs=2) as gscpool,
            tc.tile_pool(name="wp", bufs=2) as wpool,
            tc.tile_pool(name="ups", bufs=2, space="PSUM") as upool,
            tc.tile_pool(name="sps", bufs=1, space="PSUM") as spool,
        ):
            # ---------------- constants ----------------
            ones_f = cpool.tile([128, 1], F32, tag="ones_f", name="ones_f")
            nc.vector.memset(ones_f[:], 1.0)
            ones_b = cpool.tile([128, 1], BF16, tag="ones_b", name="ones_b")
            nc.vector.memset(ones_b[:], 1.0)
            mA = cpool.tile([128, 1], BF16, tag="mA", name="mA")
            nc.scalar.dma_start(mA[:], maskA[:, :])
            mB = cpool.tile([128, 1], BF16, tag="mB", name="mB")
            nc.scalar.dma_start(mB[:], maskB[:, :])
            biasE = cpool.tile([128, 1], F32, tag="biasE", name="biasE")
            nc.vector.memset(biasE[:], -ALPHA)

            # gold accumulator columns (one per piece)
            acc = cpool.tile([128, 32], F32, tag="acc", name="acc")

            # ---------------- E^T = exp(trans)^T  (from transT in blob) ----
            tr_raw = rawpool.tile([128, 4 * K], BF16, tag="tr_raw", name="tr_raw")
            nc.scalar.dma_start(
                tr_raw[:, :].rearrange("p (j c) -> p j c", j=4),
                blob[:, S2:].rearrange("(j p) c -> p j c", p=128))
            et = [etpool.tile([128, 2 * K], BF16, tag=f"et{kp}", name=f"et{kp}")
                  for kp in range(2)]
            for kp in range(2):
                nc.scalar.activation(et[kp][:], tr_raw[:, 2 * K * kp:2 * K * (kp + 1)],
                                     mybir.ActivationFunctionType.Exp)

            def et_sl(kt, jt):
                return et[kt // 2][:, K * (kt % 2) + 128 * jt:K * (kt % 2) + 128 * (jt + 1)]

            # ---------------- obs slices: chunked DMA + exp into i-major dexp
            # raw4 chunk layout: raw4[p, jt*cw + c] = blob[jt*128+p, w0+c]
            # dexp pair tiles: dexp[pp][j_local, jl*S2 + col], jt = 2*pp + jl
            dexp = [dxpool.tile([128, 2 * S2], BF16, tag=f"dexp{pp}", name=f"dexp{pp}")
                    for pp in range(2)]
            raw4s = []
            u4s = []
            cc0 = 0
            for gi, ncc in enumerate(CHUNKS):
                w0, w1 = cc0 * RW, (cc0 + ncc) * RW
                cw = w1 - w0
                raw4 = rawpool.tile([128, 4 * cw], BF16, tag=f"raw{gi}",
                                    name=f"raw{gi}")
                nc.sync.dma_start(
                    raw4[:, :].rearrange("p (j c) -> p j c", j=4),
                    blob[:, w0:w1].rearrange("(j p) c -> p j c", p=128))
                u4 = rawpool.tile([128, 4 * cw], BF16, tag=f"u{gi}", name=f"u{gi}")
                nc.gpsimd.dma_start(
                    u4[:, :].rearrange("p (j c) -> p j c", j=4),
                    gmask[:, w0:w1].rearrange("(j p) c -> p j c", p=128))
                raw4s.append(raw4)
                u4s.append(u4)
                for pp in range(2):
                    nc.scalar.activation(
                        dexp[pp][:, :].rearrange("q (j s) -> q j s", j=2)[:, :, w0:w1],
                        raw4[:, 2 * cw * pp:2 * cw * (pp + 1)].rearrange(
                            "q (j c) -> q j c", j=2),
                        mybir.ActivationFunctionType.Exp, bias=biasE[:])
                cc0 += ncc

            # gmask for the transT region
            u_tr = rawpool.tile([128, 4 * K], BF16, tag="u_tr", name="u_tr")
            nc.gpsimd.dma_start(
                u_tr[:, :].rearrange("p (j c) -> p j c", j=4),
                gmask[:, S2:].rearrange("(j p) c -> p j c", p=128))

            # gold pieces: (src_tile, u_tile, col0, cols, acc_col)
            pieces = []
            for gi, ncc in enumerate(CHUNKS):
                W4 = 4 * ncc * RW
                np_ = (W4 + PIECE - 1) // PIECE
                step = (W4 + np_ - 1) // np_
                for k in range(np_):
                    c0 = k * step
                    pieces.append((raw4s[gi], u4s[gi], c0, min(step, W4 - c0)))
            for k in range(2):
                pieces.append((tr_raw, u_tr, k * 1024, 1024))
            assert len(pieces) <= 32

            def emit_gold_piece(pi):
                src_t, u_t, c0, cols = pieces[pi]
                sc = gscpool.tile([128, PIECE], BF16, tag="gsc", name="gsc")
                nc.vector.tensor_mul(sc[:, :cols], src_t[:, c0:c0 + cols],
                                     u_t[:, c0:c0 + cols])
                nc.scalar.activation(sc[:, :cols], sc[:, :cols],
                                     mybir.ActivationFunctionType.Copy,
                                     accum_out=acc[:, pi:pi + 1])

            # ---------------- state init ----------------
            w = [wpool.tile([128, 2 * R], BF16, tag=f"w{pp}", name=f"w{pp}")
                 for pp in range(2)]
            for pp in range(2):
                nc.vector.memset(w[pp][:], 1.0)

            ls_sb = cpool.tile([1, R], F32, tag="ls_sb", name="ls_sb")
            le_sb = cpool.tile([1, R], F32, tag="le_sb", name="le_sb")

            # ---------------- main recurrence ----------------
            for i in range(1, NSTEP + 1):
                qq, cc = (i - 1) // 16, (i - 1) % 16
                off = cc * RW + qq

                u = [upool.tile([128, 2 * R], F32, tag=f"u{pp}", name=f"u{pp}")
                     for pp in range(2)]
                # kt 0/1 first (consume only w[0]), then kt 2/3 (w[1]):
                # decouples next step's first 8 MMs from this step's last TT.
                # One accumulation group per pair-bank: start on its first MM,
                # stop on its last (PSUM pending-zero gives first-touch
                # overwrite semantics for the jl=1 half).
                for kh in range(2):
                    for pp in range(2):
                        for jl in range(2):
                            jt = 2 * pp + jl
                            for kt in (2 * kh, 2 * kh + 1):
                                nc.tensor.matmul(
                                    u[pp][:, R * jl:R * (jl + 1)],
                                    et_sl(kt, jt),
                                    w[kt // 2][:, R * (kt % 2):R * (kt % 2 + 1)],
                                    start=(kh == 0 and jl == 0 and kt == 0),
                                    stop=(kh == 1 and jl == 1 and kt == 3))

                wn = [wpool.tile([128, 2 * R], BF16, tag=f"w{pp}", name=f"w{pp}")
                      for pp in range(2)]
                for pp in range(2):
                    nc.vector.tensor_mul(
                        wn[pp][:, :].rearrange("q (j s) -> q j s", j=2),
                        u[pp][:, :].rearrange("q (j s) -> q j s", j=2),
                        dexp[pp][:, :].rearrange("q (j s) -> q j s", j=2)
                        [:, :, off:off + R])
                w = wn

                if i == B:
                    # reinit sub-chunk state column 0 (core 0 only via masks)
                    for pp in range(2):
                        for jl in range(2):
                            c0 = R * jl
                            nc.vector.tensor_mul(w[pp][:, c0:c0 + 1],
                                                 w[pp][:, c0:c0 + 1], mA[:])
                            nc.vector.tensor_add(w[pp][:, c0:c0 + 1],
                                                 w[pp][:, c0:c0 + 1], mB[:])
                if i == B or i == NSTEP:
                    sig = spool.tile([1, R], F32, tag="sig", name="sig")
                    for kt in range(4):
                        nc.tensor.matmul(sig[:], ones_b[:],
                                         w[kt // 2][:, R * (kt % 2):R * (kt % 2 + 1)],
                                         start=(kt == 0), stop=(kt == 3))
                    dst = ls_sb if i == B else le_sb
                    nc.scalar.activation(dst[:], sig[:],
                                         mybir.ActivationFunctionType.Ln)

                # interleave one gold piece per step (keeps DVE/ACT slack use
                # spread so the recurrence is never starved)
                if i - 1 < len(pieces):
                    emit_gold_piece(i - 1)

            for pi in range(NSTEP, len(pieces)):
                emit_gold_piece(pi)

            # ---------------- forward partial ----------------
            diff = cpool.tile([1, R], F32, tag="diff", name="diff")
            nc.vector.tensor_sub(diff[:], le_sb[:], ls_sb[:])
            fwd_red = cpool.tile([1, 1], F32, tag="fwd_red", name="fwd_red")
            nc.vector.tensor_reduce(fwd_red[:], diff[:],
                                    axis=mybir.AxisListType.X,
                                    op=mybir.AluOpType.add)

            # ---------------- gold tail ----------------
            gvec = cpool.tile([128, 1], F32, tag="gvec", name="gvec")
            nc.vector.tensor_reduce(gvec[:], acc[:, :len(pieces)],
                                    axis=mybir.AxisListType.X,
                                    op=mybir.AluOpType.add)
            gold_ps = spool.tile([1, 1], F32, tag="gold_ps", name="gold_ps")
            nc.tensor.matmul(gold_ps[:], gvec[:],
                             ones_f[:], start=True, stop=True)

            # ---------------- output ----------------
            out_sb = cpool.tile([1, 4], F32, tag="out_sb", name="out_sb")
            nc.vector.memset(out_sb[:], 0.0)
            nc.vector.tensor_copy(out_sb[:, 0:1], fwd_red[:])
            nc.vector.tensor_copy(out_sb[:, 1:2], gold_ps[:])
            nc.sync.dma_start(out[:, :], out_sb[:])

    nc.compile()
    return nc


_NC_CACHE = None


def _get_nc():
    global _NC_CACHE
    if _NC_CACHE is None:
        _NC_CACHE = _build_nc()
    return _NC_CACHE


def _packedcol(u):
    return (u % 16) * RW + u // 16


def make_in_maps(observes, tags, transitions):
    observes = np.ascontiguousarray(np.asarray(observes, dtype=np.float32))
    transitions = np.ascontiguousarray(np.asarray(transitions, dtype=np.float32))
    tags = np.asarray(tags).astype(np.int64)
    assert observes.shape == (K, T) and transitions.shape == (K, K)

    transT = transitions.T.astype(np.float32)
    in_maps = []
    for c in range(NCORES):
        lo = c * R * L - B
        sl = np.zeros((K, S2), np.float32)
        src_lo = max(lo, 0)
        sl[:, src_lo - lo:S] = observes[:, src_lo:c * R * L + R * L]
        # pack i-major: packed[k, cc*RW + r'] = sl[k, 16*r' + cc]
        packed = sl.reshape(K, RW, 16).transpose(0, 2, 1).reshape(K, S2)
        blob = np.ascontiguousarray(
            np.concatenate([packed, transT], axis=1)).astype(ml_dtypes.bfloat16)

        mA = np.ones((128, 1), np.float32)
        mB = np.zeros((128, 1), np.float32)
        if c == 0:
            mA[:] = 0.0
            mB[:] = 1.0

        # gold mask: counts in the SAME layout as blob.
        # gold_c = sum_{i in core range, i<T-1} trans[nxt,cur] + obs[nxt, i]
        q = np.arange(GN)
        idx = c * GN + q
        valid = idx < T - 1
        qv = q[valid]
        nxt = tags[np.minimum(idx + 1, T - 1)].astype(np.int64)[valid]
        cur = tags[idx].astype(np.int64)[valid]
        U = np.zeros((K, BW), np.float32)
        np.add.at(U, (nxt, _packedcol(qv + B)), 1.0)
        np.add.at(U, (cur, S2 + nxt), 1.0)

        in_maps.append({
            "blob": blob,
            "gmask": np.ascontiguousarray(U).astype(ml_dtypes.bfloat16),
            "maskA": mA.astype(ml_dtypes.bfloat16),
            "maskB": mB.astype(ml_dtypes.bfloat16),
        })
    return in_maps


def combine(results):
    fwd = 0.0
    gold = 0.0
    for c in range(NCORES):
        o = results[c]["out"]
        fwd += float(o[0, 0])
        gold += float(o[0, 1])
    loss = fwd + T * ALPHA + np.log(512.0) - gold
    return np.float32(loss)


def run(in_maps, trace=False):
    nc = _get_nc()
    res = run_bass_kernel_spmd(nc, in_maps, list(range(NCORES)), trace=trace)
    return res


def kernel(observes, tags, transitions, length):
    assert int(length) == T
    in_maps = make_in_maps(observes, tags, transitions)
    res = run(in_maps)
    return combine(res.results)
